# revision 3
# baseline (speedup 1.0000x reference)
"""Trainium2 Bass kernel for BatchedCrossAttentionXSMM.

Reference computation (B=1, NQ=NK=2048, A=M=1024, H=16, KD=VD=64):
    q = (q_data @ query_w + query_b) * kd^-0.5      [Q, H, KD]
    k = m_data @ key_w                               [K, H, KD]
    v = m_data @ value_w                             [K, H, VD]
    logits = q k^T + bias                            [H, Q, K]
    w = softmax(logits, axis=-1)
    out = sigmoid(q_data @ gating_w) * (w @ v)       [Q, H, VD]

Sharding: tensor-parallel over heads -- 2 heads per NeuronCore, 8 cores.

Pipeline (v2): all inputs are streamed in block granularity so compute
starts ~1 block in.  m/q are host-pre-blocked into 512-token chunks;
exp(bias)^T streams in 1 MB sub-chunks on its own DMA queue into a ring.
K/V projections for token-block tb are interleaved with the attention
kt-loop of query-quarter 0; Q/gating projections for quarter qq are
deferred into the (PE-idle) slack of the preceding quarter's kt-loop.

On-device layout: logits are computed transposed ([k, q]) as K Q^T; exp()
lands E^T in SBUF in the layout the PV matmul needs; a [v | 1] stationary
operand produces both weighted values and softmax denominators in one PE
pass.  exp(bias)^T comes bf16 from the host and folds multiplicatively on
the DVE.  The two heads' logits matmuls are row-group packed (contraction
64 each) so they run concurrently on the PE.
"""

import re
import sys

for _p in ("/opt/trn_rl_repo",):
    if _p not in sys.path:
        sys.path.insert(0, _p)

import ml_dtypes
import numpy as np

import concourse.bass as bass
import concourse.mybir as mybir
import concourse.tile as tile
from concourse.bass_utils import run_bass_kernel_spmd
from concourse.masks import make_identity

BF16 = ml_dtypes.bfloat16
dt = mybir.dt

NCORES = 8
H_PER_CORE = 2
NQ = NK = 2048
A_DIM = 1024
KD = VD = 64
HC = H_PER_CORE * KD  # 128
SCALE = float(KD) ** -0.5
P = 128
AT = A_DIM // P  # 8 a-subtiles
QT = NQ // P     # 16 token tiles
KT = NK // P
NTB = 4          # 512-token blocks for m/q streaming
NSUB = 4         # bias sub-chunks per q-quarter (4 kt each)


# --- Tile tail-drain patch -------------------------------------------------
# The walrus build in this image caps sem-waits per instruction at 2; Tile's
# kernel-tail drain attaches one wait per live semaphore to a single Drain,
# which fails codegen ("Too many sync wait commands").  Spread the waits over
# a chain of SP nops (1 wait each) before the drain instead.
def _patched_drain_and_barrier(self, tick_clock, wait_clock):
    nc = self.nc
    gc = tick_clock.global_clock
    vals = [int(v) for v in re.findall(r"\d+", repr(gc))]
    alloc = self.sems.allocated()
    waits = []
    for proc, sem in alloc.items():
        v = vals[proc] if proc < len(vals) else 0
        if v > 0:
            mult = 16 if "DMA" in sem.name else 1
            waits.append((sem, v * mult))
    for sem, val in waits:
        nc.sync.nop(nofuse=True).wait_op(sem, val, "sem-ge")
    nc.sync.drain()
    nc.all_engine_barrier()
    popped = nc._tile_sem_poison_stack.pop()
    assert popped is self._sem_poison
    nc.clear_and_free_semaphores(list(self.sems.allocated().values()))
    nc.all_engine_barrier()


tile.TileContext._drain_and_barrier = _patched_drain_and_barrier


# --- BIR wait-splitting pass ----------------------------------------------
# Tile's wait assignment can attach 3+ semaphore waits to a single
# instruction; this walrus build encodes at most 2 wait commands per
# instruction.  Rewrite the serialized BIR: hoist excess waits onto
# preceding EventSemaphore instructions on the same engine.
_MAXW = 1
_orig_to_json_bytes = bass.Bass.to_json_bytes


def _to_json_bytes_split_waits(self):
    import json

    data = json.loads(_orig_to_json_bytes(self))
    ctr = 0
    for fn in data.get("functions", []):
        for bb in fn.get("blocks", []):
            newl = []
            for ins in bb["instructions"]:
                si = ins.get("sync_info")
                if si and si.get("on_wait") and len(si["on_wait"]) > _MAXW:
                    waits = si["on_wait"]
                    extra, keep = waits[:-_MAXW], waits[-_MAXW:]
                    for i in range(0, len(extra), _MAXW):
                        ctr += 1
                        newl.append({
                            "debug": ins.get("debug", 0),
                            "engine": ins["engine"],
                            "ins": [],
                            "outs": [],
                            "name": f"{ins['name']}-wsplit{ctr}",
                            "opcode": "EventSemaphore",
                            "sync_info": {
                                "on_update": [],
                                "on_wait": extra[i:i + _MAXW],
                            },
                        })
                    si["on_wait"] = keep
                newl.append(ins)
            bb["instructions"] = newl
    return json.dumps(data).encode()


bass.Bass.to_json_bytes = _to_json_bytes_split_waits


# --- device program --------------------------------------------------------
def build_nc():
    nc = bass.Bass()
    f32, bf16 = dt.float32, dt.bfloat16
    Exp = mybir.ActivationFunctionType.Exp
    Tanh = mybir.ActivationFunctionType.Tanh

    # m/q host-pre-blocked: row = tb*128 + p, dims [at, 512tok]
    mB_d = nc.dram_tensor("mB", [NTB * P, AT, 512], bf16, kind="ExternalInput")
    qB_d = nc.dram_tensor("qB", [NTB * P, AT, 512], bf16, kind="ExternalInput")
    # exp(bias)^T sub-chunks: row = (qq*NSUB + g)*128 + p, dims [h, j(kt%4), 512q]
    ebs_d = nc.dram_tensor("ebs", [4 * NSUB * P, H_PER_CORE, 4, 512], bf16,
                           kind="ExternalInput")
    wq_d = nc.dram_tensor("wq", [P, AT, HC], bf16, kind="ExternalInput")
    wk_d = nc.dram_tensor("wk", [P, AT, HC], bf16, kind="ExternalInput")
    wv_d = nc.dram_tensor("wv", [P, AT, HC], bf16, kind="ExternalInput")
    wg_d = nc.dram_tensor("wg", [P, AT, HC], bf16, kind="ExternalInput")
    bq_d = nc.dram_tensor("bq", [HC, 1], f32, kind="ExternalInput")
    # output: row = qq*128 + p, dims [qt4, hc]
    o_d = nc.dram_tensor("o", [4 * P, 4, HC], f32, kind="ExternalOutput")

    with tile.TileContext(nc) as tc:
        with (
            tc.tile_pool(name="consts", bufs=1) as consts,
            tc.tile_pool(name="mp", bufs=3) as mp,
            tc.tile_pool(name="qp", bufs=2) as qp,
            tc.tile_pool(name="bp", bufs=6) as bp,
            tc.tile_pool(name="etp", bufs=3) as etp,
            tc.tile_pool(name="gp", bufs=2) as gp,
            tc.tile_pool(name="wsbp", bufs=2) as wsbp,
            tc.tile_pool(name="smallp", bufs=2) as smallp,
            tc.tile_pool(name="outp", bufs=2) as outp,
            tc.tile_pool(name="pL", bufs=3, space="PSUM") as pL,
            tc.tile_pool(name="pW", bufs=2, space="PSUM") as pW,
        ):
            # ---- constants / warmup ----
            id_bf = consts.tile([P, P], bf16, tag="id_bf")
            make_identity(nc, id_bf)
            id_f32 = consts.tile([P, P], f32, tag="id_f32")
            make_identity(nc, id_f32)
            warm_sb = consts.tile([P, 512], bf16, tag="warm_sb")
            nc.vector.memset(warm_sb, 0.0)
            # preload the exp table set at t~0 so the ~2.7us load is off the
            # critical path
            tab_out = consts.tile([P, 16], f32, tag="tab_out")
            nc.scalar.activation(out=tab_out, in_=warm_sb[:, 0:16], func=Exp)
            # keep the PE busy until the first m block lands (HAM un-throttle)
            for i in range(16):
                wps_warm = pL.tile([P, 2, 512], f32, tag="pl",
                                   name=f"warm{i}")
                nc.tensor.matmul(wps_warm[:, 0, :], lhsT=id_bf, rhs=warm_sb,
                                 start=True, stop=True)

            # ---- DMA issue (program order ~ priority) ----
            # sync HWDGE: the 16 bias sub-chunks (1.05 MB each) into a ring
            bias_t = {}
            for qq in range(4):
                for g in range(NSUB):
                    t = bp.tile([P, H_PER_CORE, 4, 512], bf16, tag="bias",
                                name=f"bias{qq}_{g}")
                    r = (qq * NSUB + g) * P
                    nc.sync.dma_start(out=t, in_=ebs_d[r:r + P, :, :, :])
                    bias_t[(qq, g)] = t
            # scalar HWDGE: m blocks
            mblk = []
            for tb in range(NTB):
                t = mp.tile([P, AT, 512], bf16, tag="m", name=f"m{tb}")
                nc.scalar.dma_start(out=t, in_=mB_d[tb * P:(tb + 1) * P, :, :])
                mblk.append(t)
            # gpsimd SWDGE: weights, bq, q blocks (and output later)
            w_sb = {}
            for name, d in (("wk", wk_d), ("wv", wv_d), ("wq", wq_d),
                            ("wg", wg_d)):
                t = consts.tile([P, AT, HC], bf16, tag=f"{name}_sb")
                nc.gpsimd.dma_start(out=t, in_=d[:, :, :])
                w_sb[name] = t
            bq_sb = consts.tile([HC, 1], f32, tag="bq_sb")
            nc.gpsimd.dma_start(out=bq_sb, in_=bq_d[:, :])
            qblk = []
            for tb in range(NTB):
                t = qp.tile([P, AT, 512], bf16, tag="q", name=f"q{tb}")
                nc.gpsimd.dma_start(out=t, in_=qB_d[tb * P:(tb + 1) * P, :, :])
                qblk.append(t)

            # ---- persistent SBUF ----
            kT2 = consts.tile([HC, NK], bf16, tag="kT2")
            qT2 = consts.tile([HC, NQ], bf16, tag="qT2")
            vT2 = consts.tile([HC, NK], bf16, tag="vT2")
            v_sb = consts.tile([P, H_PER_CORE, KT, VD + 1], bf16, tag="v_sb")
            nc.vector.memset(v_sb[:, :, :, VD:VD + 1], 1.0)
            gate_sb = consts.tile([P, QT, HC], f32, tag="gate_sb")

            # ---- helpers ----
            def proj_tb(tb):
                """K/V projections + V transpose for m token-block tb."""
                tbs = slice(tb * 512, (tb + 1) * 512)
                psK = pL.tile([P, 512], f32, tag="pl", name=f"psK{tb}")
                psV = pL.tile([P, 512], f32, tag="pl", name=f"psV{tb}")
                for at in range(AT):
                    st, sp = (at == 0), (at == AT - 1)
                    nc.tensor.matmul(psK, lhsT=w_sb["wk"][:, at, :],
                                     rhs=mblk[tb][:, at, :], start=st, stop=sp)
                for at in range(AT):
                    st, sp = (at == 0), (at == AT - 1)
                    nc.tensor.matmul(psV, lhsT=w_sb["wv"][:, at, :],
                                     rhs=mblk[tb][:, at, :], start=st, stop=sp)
                nc.vector.tensor_copy(out=kT2[:, tbs], in_=psK)
                nc.vector.tensor_copy(out=vT2[:, tbs], in_=psV)
                for j in range(4):
                    kt = tb * 4 + j
                    tps = pL.tile([P, P], f32, tag="pl", name=f"vtp{kt}")
                    nc.tensor.matmul(tps, lhsT=vT2[:, kt * P:(kt + 1) * P],
                                     rhs=id_bf, start=True, stop=True)
                    nc.vector.tensor_copy(
                        out=v_sb[:, :, kt, 0:VD],
                        in_=tps.rearrange("p (h c) -> p h c", h=H_PER_CORE),
                    )

            def proj_q(qq):
                """Q projection for quarter qq -> qT2[:, qq*512:...]."""
                qs = slice(qq * 512, (qq + 1) * 512)
                psQ = pL.tile([P, 512], f32, tag="pl", name=f"psQ{qq}")
                for at in range(AT):
                    st, sp = (at == 0), (at == AT - 1)
                    nc.tensor.matmul(psQ, lhsT=w_sb["wq"][:, at, :],
                                     rhs=qblk[qq][:, at, :], start=st, stop=sp)
                nc.vector.tensor_scalar(
                    out=qT2[:, qs], in0=psQ, scalar1=bq_sb, scalar2=SCALE,
                    op0=mybir.AluOpType.add, op1=mybir.AluOpType.mult,
                )

            def proj_g(qq):
                """Gating projection + sigmoid + transpose for quarter qq."""
                psG = pL.tile([P, 512], f32, tag="pl", name=f"psG{qq}")
                for at in range(AT):
                    st, sp = (at == 0), (at == AT - 1)
                    nc.tensor.matmul(psG, lhsT=w_sb["wg"][:, at, :],
                                     rhs=qblk[qq][:, at, :], start=st, stop=sp)
                # sigmoid(x) = 0.5 + 0.5*tanh(x/2): tanh shares the exp ACT
                # table set, so no table reload.
                gth = gp.tile([P, 512], f32, tag="gth", name=f"gth{qq}")
                nc.scalar.activation(out=gth, in_=psG, func=Tanh, scale=0.5)
                gt2 = gp.tile([P, 512], bf16, tag="gt2", name=f"gt2{qq}")
                nc.vector.tensor_scalar(
                    out=gt2, in0=gth, scalar1=0.5, scalar2=0.5,
                    op0=mybir.AluOpType.mult, op1=mybir.AluOpType.add,
                )
                gps = pL.tile([P, 4, P], f32, tag="pl", name=f"gps{qq}")
                for qb in range(4):
                    nc.tensor.matmul(gps[:, qb, :],
                                     lhsT=gt2[:, qb * P:(qb + 1) * P],
                                     rhs=id_bf, start=True, stop=True)
                nc.vector.tensor_copy(out=gate_sb[:, qq * 4:(qq + 1) * 4, :],
                                      in_=gps)

            def fixup(qq, wps):
                """Transpose [v.w | sums]^T back to [q, c], divide, gate,
                and DMA the quarter out."""
                out_t = outp.tile([P, 4, HC], f32, tag="out", name=f"out{qq}")
                for h in range(H_PER_CORE):
                    hs = slice(h * KD, (h + 1) * KD)
                    wsb = wsbp.tile([P, 512], f32, tag="wsb",
                                    name=f"wsb{qq}_{h}")
                    nc.vector.tensor_copy(out=wsb[0:VD + 1, :],
                                          in_=wps[h][0:VD + 1, :])
                    tp4 = pL.tile([P, 4, VD + 1], f32, tag="pl",
                                  name=f"tp4_{qq}_{h}")
                    for qb in range(4):
                        nc.tensor.matmul(
                            tp4[:, qb, :],
                            lhsT=wsb[0:VD + 1, qb * P:(qb + 1) * P],
                            rhs=id_f32[0:VD + 1, 0:VD + 1],
                            is_transpose=True, start=True, stop=True,
                        )
                    rec4 = smallp.tile([P, 4], f32, tag="rec",
                                       name=f"rec{qq}_{h}")
                    nc.vector.reciprocal(out=rec4, in_=tp4[:, :, VD])
                    tmp4 = smallp.tile([P, 4, VD], f32, tag="tmp",
                                       name=f"tmp{qq}_{h}")
                    for qb in range(4):
                        nc.vector.tensor_scalar_mul(
                            out=tmp4[:, qb, :], in0=tp4[:, qb, 0:VD],
                            scalar1=rec4[:, qb:qb + 1])
                    nc.vector.tensor_mul(
                        out=out_t[:, :, hs],
                        in0=tmp4,
                        in1=gate_sb[:, qq * 4:(qq + 1) * 4, hs],
                    )
                nc.gpsimd.dma_start(out=o_d[qq * P:(qq + 1) * P, :, :],
                                    in_=out_t)

            # ---- main pipeline ----
            prev = None  # (qq, wps) awaiting fixup
            for qq in range(4):
                qs = slice(qq * 512, (qq + 1) * 512)
                if qq == 0:
                    proj_tb(0)
                    proj_q(0)
                if prev is not None:
                    fixup(*prev)
                    prev = None
                wps = [pW.tile([P, 512], f32, tag="pw", name=f"wps{qq}_{h}")
                       for h in range(H_PER_CORE)]
                for kt in range(KT):
                    if qq == 0 and kt % 4 == 2 and kt < 14:
                        proj_tb(kt // 4 + 1)
                    if kt == 10:
                        proj_g(qq)
                    if kt == 12 and qq < 3:
                        proj_q(qq + 1)
                    ks = slice(kt * P, (kt + 1) * P)
                    lpp = pL.tile([P, H_PER_CORE, 512], f32, tag="pl",
                                  name=f"lpp{qq}_{kt}")
                    for h in range(H_PER_CORE):
                        hs = slice(h * KD, (h + 1) * KD)
                        nc.tensor.matmul(
                            lpp[:, h, :],
                            lhsT=kT2[hs, ks], rhs=qT2[hs, qs],
                            start=True, stop=True,
                        )
                    etr = etp.tile([P, H_PER_CORE, 512], bf16, tag="etr",
                                   name=f"etr{qq}_{kt}")
                    nc.scalar.activation(out=etr, in_=lpp, func=Exp)
                    et = etp.tile([P, H_PER_CORE, 512], bf16, tag="et",
                                  name=f"et{qq}_{kt}")
                    nc.vector.tensor_mul(
                        out=et, in0=etr,
                        in1=bias_t[(qq, kt // 4)][:, :, kt % 4, :])
                    for h in range(H_PER_CORE):
                        nc.tensor.matmul(
                            wps[h][0:VD + 1, :],
                            lhsT=v_sb[:, h, kt, :],
                            rhs=et[:, h, :],
                            start=(kt == 0), stop=(kt == KT - 1),
                        )
                prev = (qq, wps)
            fixup(*prev)

    return nc


_NC = None


def _get_nc():
    global _NC
    if _NC is None:
        _NC = build_nc()
    return _NC


# --- host side -------------------------------------------------------------
def prepare_in_maps(q_data, m_data, batched_bias, query_w, query_b, key_w,
                    value_w, gating_w):
    q = np.asarray(q_data, np.float32)[0]          # [NQ, A]
    m = np.asarray(m_data, np.float32)[0]          # [NK, A]
    bias = np.asarray(batched_bias, np.float32)[0]  # [H, NQ, NK]
    bq = np.asarray(query_b, np.float32)[0]        # [H, KD]

    # [A, N] -> token blocks [tb*128+p, at, 512]
    def blockify(x):
        xT = x.T  # [A, N]
        b = xT.reshape(AT, P, NTB, 512).transpose(2, 1, 0, 3)
        return np.ascontiguousarray(b.reshape(NTB * P, AT, 512)).astype(BF16)

    qB = blockify(q)
    mB = blockify(m)

    def wslice(w, c):
        w = np.asarray(w, np.float32)
        ws = w[:, 2 * c:2 * c + 2, :].reshape(A_DIM, HC)
        ws = ws.reshape(AT, P, HC).transpose(1, 0, 2)
        return np.ascontiguousarray(ws).astype(BF16)

    in_maps = []
    for c in range(NCORES):
        # exp(bias)^T  [h, k, q] -> sub-chunks [(qq*4+g)*128+p, h, j, 512]
        ebT = np.exp(bias[2 * c:2 * c + 2].transpose(0, 2, 1))
        # dims: [h, g, j, p, qq, q'] -> [qq, g, p, h, j, q']
        eb = ebT.reshape(H_PER_CORE, 4, 4, P, 4, 512).transpose(4, 1, 3, 0, 2, 5)
        ebs = np.ascontiguousarray(
            eb.reshape(4 * NSUB * P, H_PER_CORE, 4, 512)).astype(BF16)
        in_maps.append({
            "qB": qB,
            "mB": mB,
            "ebs": ebs,
            "wq": wslice(query_w, c),
            "wk": wslice(key_w, c),
            "wv": wslice(value_w, c),
            "wg": wslice(gating_w, c),
            "bq": np.ascontiguousarray(bq[2 * c:2 * c + 2].reshape(HC, 1)),
        })
    return in_maps


def gather_out(results):
    # o: [qq*128+p, qt4, hc] -> [NQ, H_PER_CORE, VD]
    parts = []
    for r in results:
        o = np.asarray(r["o"]).reshape(4, P, 4, HC)
        o = o.transpose(0, 2, 1, 3).reshape(NQ, H_PER_CORE, VD)
        parts.append(o)
    return np.concatenate(parts, axis=1)[None].astype(np.float32)


def kernel(**inputs):
    in_maps = prepare_in_maps(**inputs)
    res = run_bass_kernel_spmd(_get_nc(), in_maps, core_ids=list(range(NCORES)))
    return gather_out(res.results)
